# revision 12
# baseline (speedup 1.0000x reference)
"""Trainium2 Bass kernel for AetherLoss: chamfer(recon_x, x) + beta*KL(mu, logvar).

Strategy ("banded KNN", data-parallel over batch B=8 across 8 NeuronCores):

Host prep: both point clouds are sorted by their z coordinate; the fp32->
3x-bf16 augmented operands are built for BOTH directions (AX/AY for
x-query-vs-y and BX/BY for y-query-vs-x), replicated into the four 32-row
PE strips, [128, 4096] bf16 each.

Instead of the full 4096x4096 distance matrix, each 128-query tile only
computes distances to a W=512 window of candidates centered at its sorted
position (one PSUM bank per tile; groups of 4 tiles run as concurrent
row-packed K=24 matmuls in the 4 PE strips).  That is 8x less PSUM traffic
than the all-pairs baseline; per-row nearest-neighbor correctness outside
the band is certified on the host with an exact geometric bound (in-band
min <= squared z-gap to the band edge implies no outside point can win),
and rows failing the certificate (~25%) are recomputed exactly on the host
- the same rescue machinery (and a similar rescue share) as the all-pairs
softmin baseline this replaces.

Per tile the row reduction is either an exact DVE reduce_max over the
negated distances (tail tiles, fp32, no softmin bias) or a ScalarE
exp(S*-d) ACTIVATE whose fused accumulator emits the softmin row sum in
the same pass (center tiles) - the mix balances ScalarE ~= DVE.  The
column direction is handled by the transposed (BX/BY) tiles the same way,
so the all-pairs design's column accumulators, their memsets, and the
per-half DVE max folds disappear entirely; outputs shrink from ~3MB to
33KB per core.  KL runs inside the input-DMA wait window at boot; the
x-direction results are DMA-ed out at the loop midpoint.

Host combine (numpy, float64): exact tiles give -min directly; softmin
tiles give -ln(rowsum)/S with S=1600; rows with rowsum underflow or a
failed band certificate are recomputed exactly from the raw inputs (one
small fp32 gemm per core per direction).  Validated end-to-end in numpy
against the fp32 reference at rel err ~1.7e-4 (matching the all-pairs
baseline's measured error).
"""

import numpy as np
import ml_dtypes
from contextlib import ExitStack

B, D, N = 8, 3, 4096
LATENT = 256
NCORES = 8
BETA = 1.0

K = 24              # augmented contraction size
PT = 128            # query tile size
NT = N // PT        # 32 query tiles per direction
W = 512             # candidate band width (1 PSUM bank)

S = 1600.0          # softmin sharpness
LN_THRESH = -60.0   # underflow threshold on ln(rowsum)
SC_MARGIN = 2e-3    # certificate margin for softmin tiles
EX_MARGIN = 5e-4    # certificate margin for exact tiles (bf16 matmul noise)

# Center tiles go through ScalarE softmin, tail tiles through exact DVE
# reduce_max (balances ScalarE ~= DVE; 13 + 19 tiles per direction).
SC_TILES = frozenset(range(8, 21))

bf16 = ml_dtypes.bfloat16

_cache = {}


def band_lo(pt):
    return int(np.clip(pt * PT + PT // 2 - W // 2, 0, N - W))


def _split3(v):
    h = v.astype(bf16)
    m = (v - h.astype(np.float64)).astype(bf16)
    l = (v - h.astype(np.float64) - m.astype(np.float64)).astype(bf16)
    return h, m, l


def build_aug(x, y):
    """x (queries), y (candidates): [3, N] float64 -> AX, AY [128, N] bf16
    with AX[:, i] . AY[:, j] = -(||x_i - y_j||^2), replicated into the four
    32-row PE strips."""
    axh, axm, axl = _split3(2.0 * x)
    yh, ym, yl = _split3(y)
    x2h, x2m, x2l = _split3(-(x * x).sum(0)[None, :])
    y2h, y2m, y2l = _split3(-(y * y).sum(0)[None, :])
    ones = np.ones((3, x.shape[1]), dtype=bf16)
    AX = np.concatenate([
        axh, axh, axm, axh, axl, axm,
        np.concatenate([x2h, x2m, x2l], 0), ones], 0).astype(bf16)
    AY = np.concatenate([
        yh, ym, yh, yl, yh, ym,
        ones, np.concatenate([y2h, y2m, y2l], 0)], 0).astype(bf16)
    n = x.shape[1]
    AX4 = np.zeros((128, n), dtype=bf16)
    AY4 = np.zeros((128, n), dtype=bf16)
    for q in range(4):
        AX4[32 * q:32 * q + K] = AX
        AY4[32 * q:32 * q + K] = AY
    return AX4, AY4


def _build_program():
    import concourse.bass as bass
    import concourse.tile as tile
    from concourse import bacc, mybir

    f32 = mybir.dt.float32
    bf = mybir.dt.bfloat16
    MULT = mybir.AluOpType.mult

    nc = bacc.Bacc(trn_type="TRN2", debug=False, target_bir_lowering=False)

    ax = nc.dram_tensor("ax", [128, N], bf, kind="ExternalInput")
    ay = nc.dram_tensor("ay", [128, N], bf, kind="ExternalInput")
    mu = nc.dram_tensor("mu", [LATENT], f32, kind="ExternalInput")
    lv = nc.dram_tensor("lv", [LATENT], f32, kind="ExternalInput")

    o_row = nc.dram_tensor("o_row", [128, NT], f32, kind="ExternalOutput")
    o_col = nc.dram_tensor("o_col", [128, NT], f32, kind="ExternalOutput")
    o_kl = nc.dram_tensor("o_kl", [128, 1], f32, kind="ExternalOutput")

    with tile.TileContext(nc) as tc, ExitStack() as ctx:
        const = ctx.enter_context(tc.tile_pool(name="const", bufs=1))
        work = ctx.enter_context(tc.tile_pool(name="work", bufs=1))
        stg = ctx.enter_context(tc.tile_pool(name="stg", bufs=4))
        psum = ctx.enter_context(tc.tile_pool(name="psum", bufs=2, space="PSUM"))

        axs = const.tile([128, N], bf, tag="axs")
        ays = const.tile([128, N], bf, tag="ays")
        # Chunked input DMA (4 x [128, 1024] per tensor) so the first tile
        # groups only wait on the chunks they read, not the whole tensor.
        CH = 1024
        # First groups issued are 2,3 (ScalarE-heavy, tiles 8-15): they
        # read ax chunk 1 and ay chunks 0-2, so prioritize those.
        for k in (1, 2, 0, 3):
            sl = slice(k * CH, (k + 1) * CH)
            nc.sync.dma_start(axs[:, sl], ax.ap()[:, sl])
        for k in (0, 1, 2, 3):
            sl = slice(k * CH, (k + 1) * CH)
            nc.gpsimd.dma_start(ays[:, sl], ay.ap()[:, sl])

        row_t = const.tile([128, NT], f32, tag="row_t")
        col_t = const.tile([128, NT], f32, tag="col_t")

        # ---- KL term: inside the input-DMA wait window ----
        mu2d = work.tile([128, LATENT // 128], f32, tag="mu2d")
        lv2d = work.tile([128, LATENT // 128], f32, tag="lv2d")
        nc.scalar.dma_start(mu2d[:], mu.ap().rearrange("(p f) -> p f", p=128))
        nc.scalar.dma_start(lv2d[:], lv.ap().rearrange("(p f) -> p f", p=128))
        klsq = work.tile([128, LATENT // 128], f32, tag="klsq")
        klex = work.tile([128, LATENT // 128], f32, tag="klex")
        klt = work.tile([128, LATENT // 128], f32, tag="klt")
        klp = work.tile([128, 1], f32, tag="klp")
        nc.vector.tensor_tensor(klsq[:], mu2d[:], mu2d[:], op=MULT)
        nc.scalar.activation(klex[:], lv2d[:], mybir.ActivationFunctionType.Exp)
        nc.vector.tensor_tensor(klt[:], lv2d[:], klsq[:],
                                op=mybir.AluOpType.subtract)
        nc.vector.tensor_tensor(klt[:], klt[:], klex[:],
                                op=mybir.AluOpType.subtract)
        nc.vector.reduce_sum(klp[:], klt[:], axis=mybir.AxisListType.X)
        nc.sync.dma_start(o_kl.ap(), klp[:])

        # ---- main loop: 2 directions x 8 groups of 4 consecutive band
        # tiles, row-packed into the 4 PE strips.  Groups issue in an
        # order that alternates DVE-heavy (tail) and ScalarE-heavy
        # (center) groups so both engines stay fed with psum bufs=2,
        # while low-column groups go first to ride the chunked DMA.
        # The y direction reuses the same operands with the roles
        # swapped: AY stationary / AX moving gives -(d(y_i, x_j)). ----
        # Issue order pairs ScalarE-heavy groups (S) and DVE groups (D) as
        # S S D D ... so both psum buffers (even/odd positions) alternate
        # engines; consecutive DVE tiles in a group collapse into one
        # multi-bank reduce ([128, m, W] AP) to amortize per-op overhead.
        GROUP_ORDER = [2, 3, 0, 1, 4, 5, 6, 7]
        for di, (stat, mov, ost) in enumerate(
                ((axs, ays, row_t), (ays, axs, col_t))):
            for g in GROUP_ORDER:
                tiles = [4 * g, 4 * g + 1, 4 * g + 2, 4 * g + 3]
                ptile = psum.tile([128, 4 * W], f32, tag="ptile",
                                  name=f"pt{di}_{g}")
                for q, pt in enumerate(tiles):
                    lo = band_lo(pt)
                    nc.tensor.matmul(
                        ptile[:, q * W:(q + 1) * W],
                        stat[32 * q:32 * q + K, pt * PT:(pt + 1) * PT],
                        mov[32 * q:32 * q + K, lo:lo + W],
                        start=True, stop=True,
                        tile_position=(32 * q, 0),
                    )
                q = 0
                while q < 4:
                    pt = tiles[q]
                    if pt in SC_TILES:
                        ex = stg.tile([128, W], bf, tag="exh",
                                      name=f"ex{di}_{g}_{q}")
                        nc.scalar.activation(
                            ex[:], ptile[:, q * W:(q + 1) * W],
                            mybir.ActivationFunctionType.Exp, scale=S,
                            accum_out=ost[:, pt:pt + 1])
                        q += 1
                    else:
                        m = 1
                        while q + m < 4 and tiles[q + m] not in SC_TILES:
                            m += 1
                        nc.vector.reduce_max(
                            ost[:, pt:pt + m],
                            ptile[:, q * W:(q + m) * W].rearrange(
                                "p (m w) -> p m w", m=m),
                            axis=mybir.AxisListType.X)
                        q += m
            if di == 0:
                # x-direction results final: ship while y-direction computes
                nc.sync.dma_start(o_row.ap(), row_t[:])
            else:
                # tiles 16..31 (groups 4..7) finish last in GROUP_ORDER;
                # ship the first half early to shorten the exit tail
                nc.gpsimd.dma_start(o_col.ap()[:, 0:16], col_t[:, 0:16])
        nc.gpsimd.dma_start(o_col.ap()[:, 16:NT], col_t[:, 16:NT])

    nc.compile()
    return nc


def _get_nc():
    if "nc" not in _cache:
        _cache["nc"] = _build_program()
    return _cache["nc"]


def _register_ntff_hook():
    import sys, types
    if "antenv.axon_hooks" in sys.modules:
        return
    try:
        from trn_agent_boot.trn_boot import _ntff_profile_via_ctypes
        hook = _ntff_profile_via_ctypes("/opt/axon/libaxon_pjrt.so")
        mod = types.ModuleType("antenv.axon_hooks")
        mod.get_axon_ntff_profile_hook = lambda: hook
        mod.set_axon_ntff_profile_hook = lambda h: None
        sys.modules["antenv.axon_hooks"] = mod
        from concourse import bass_utils
        bass_utils.upload_artifacts = lambda tmpdir: tmpdir
    except Exception:
        pass


def _run(in_maps, trace=False):
    from concourse.bass_utils import run_bass_kernel_spmd
    if trace:
        _register_ntff_hook()
    nc = _get_nc()
    return run_bass_kernel_spmd(nc, in_maps, list(range(NCORES)), trace=trace)


def _side_vals(dev, xs_raw, ys_raw):
    """Decode one direction for one core.

    dev: [128, NT] device output (softmin rowsum for SC_TILES columns,
    -min for the rest).  xs_raw/ys_raw: [3, N] fp32 query/candidate points
    (unsorted).  Returns the mean of per-query-row min squared distances.
    """
    zx = np.argsort(xs_raw[2], kind="stable")
    zy = np.argsort(ys_raw[2], kind="stable")
    xs = xs_raw[:, zx].astype(np.float64)
    ys = ys_raw[:, zy].astype(np.float64)
    thresh = np.exp(LN_THRESH)
    vals = np.zeros(N)
    need = np.zeros(N, dtype=bool)
    dev = dev.astype(np.float64)
    for pt in range(NT):
        rows = slice(pt * PT, pt * PT + PT)
        lo = band_lo(pt)
        hi = lo + W
        zlo = ys[2, lo - 1] if lo > 0 else -np.inf
        zhi = ys[2, hi] if hi < N else np.inf
        zi = xs[2, rows]
        gap = np.minimum(zi - zlo, zhi - zi)
        gap2 = np.where(gap > 0, gap * gap, 0.0)
        v = dev[:, pt]
        if pt in SC_TILES:
            with np.errstate(divide="ignore"):
                est = np.where(v > 0, -np.log(np.maximum(v, 1e-300)) / S,
                               np.inf)
            bad = (v < thresh) | (est > gap2 - SC_MARGIN)
        else:
            est = -v
            bad = est > gap2 - EX_MARGIN
        vals[rows] = est
        need[rows] = bad
    if need.any():
        idx = np.nonzero(need)[0]
        xf = xs.astype(np.float32)
        yf = ys.astype(np.float32)
        xr = xf[:, idx]
        d = ((xr * xr).sum(0)[:, None] + (yf * yf).sum(0)[None, :]
             - 2.0 * xr.T @ yf)
        vals[idx] = d.min(1).astype(np.float64)
    return vals.mean()


def _combine(results, recon_x, x):
    """Host-side finish: decode per-tile reductions, certify bands, rescue."""
    row_total = 0.0
    col_total = 0.0
    kl_sum = 0.0
    for c in range(NCORES):
        r = results[c]
        row_total += _side_vals(r["o_row"], recon_x[c], x[c])
        col_total += _side_vals(r["o_col"], x[c], recon_x[c])
        kl_sum += r["o_kl"].astype(np.float64).sum()

    recon = (row_total + col_total) / NCORES
    kld = -0.5 * (B * LATENT * 1.0 + kl_sum) / B
    total = recon + BETA * kld
    return (np.float32(total), np.float32(recon), np.float32(kld))


def _prep_in_maps(recon_x, x, mu, logvar):
    in_maps = []
    for c in range(NCORES):
        xs = recon_x[c][:, np.argsort(recon_x[c, 2], kind="stable")]
        ys = x[c][:, np.argsort(x[c, 2], kind="stable")]
        xs = xs.astype(np.float64)
        ys = ys.astype(np.float64)
        AX, AY = build_aug(xs, ys)
        in_maps.append({"ax": AX, "ay": AY, "mu": mu[c], "lv": logvar[c]})
    return in_maps


def kernel(recon_x, x, mu, logvar, _trace=False):
    recon_x = np.ascontiguousarray(recon_x, dtype=np.float32)
    x = np.ascontiguousarray(x, dtype=np.float32)
    mu = np.ascontiguousarray(mu, dtype=np.float32)
    logvar = np.ascontiguousarray(logvar, dtype=np.float32)
    in_maps = _prep_in_maps(recon_x, x, mu, logvar)
    res = _run(in_maps, trace=_trace)
    out = _combine(res.results, recon_x, x)
    if _trace:
        return out, res
    return out


# revision 14
# speedup vs baseline: 1.2956x; 1.2956x over previous
"""Trainium2 Bass kernel for AetherLoss: chamfer(recon_x, x) + beta*KL(mu, logvar).

Strategy ("banded KNN", data-parallel over batch B=8 across 8 NeuronCores):

Host prep: both point clouds are sorted by their z coordinate; the fp32->
3x-bf16 augmented operands are built for BOTH directions (AX/AY for
x-query-vs-y and BX/BY for y-query-vs-x), replicated into the four 32-row
PE strips, [128, 4096] bf16 each.

Instead of the full 4096x4096 distance matrix, each 128-query tile only
computes distances to a W=512 window of candidates centered at its sorted
position (one PSUM bank per tile; groups of 4 tiles run as concurrent
row-packed K=24 matmuls in the 4 PE strips).  That is 8x less PSUM traffic
than the all-pairs baseline; per-row nearest-neighbor correctness outside
the band is certified on the host with an exact geometric bound (in-band
min <= squared z-gap to the band edge implies no outside point can win),
and rows failing the certificate (~25%) are recomputed exactly on the host
- the same rescue machinery (and a similar rescue share) as the all-pairs
softmin baseline this replaces.

Per tile the row reduction is either an exact DVE reduce_max over the
negated distances (tail tiles, fp32, no softmin bias) or a ScalarE
exp(S*-d) ACTIVATE whose fused accumulator emits the softmin row sum in
the same pass (center tiles) - the mix balances ScalarE ~= DVE.  The
column direction is handled by the transposed (BX/BY) tiles the same way,
so the all-pairs design's column accumulators, their memsets, and the
per-half DVE max folds disappear entirely; outputs shrink from ~3MB to
33KB per core.  KL runs inside the input-DMA wait window at boot; the
x-direction results are DMA-ed out at the loop midpoint.

Host combine (numpy, float64): exact tiles give -min directly; softmin
tiles give -ln(rowsum)/S with S=1600; rows with rowsum underflow or a
failed band certificate are recomputed exactly from the raw inputs (one
small fp32 gemm per core per direction).  Validated end-to-end in numpy
against the fp32 reference at rel err ~1.7e-4 (matching the all-pairs
baseline's measured error).
"""

import numpy as np
import ml_dtypes
from contextlib import ExitStack

B, D, N = 8, 3, 4096
LATENT = 256
NCORES = 8
BETA = 1.0

K = 24              # augmented contraction size
PT = 128            # query tile size
NT = N // PT        # 32 query tiles per direction
W = 512             # candidate band width (1 PSUM bank)

S = 1600.0          # softmin sharpness
LN_THRESH = -60.0   # underflow threshold on ln(rowsum)
SC_MARGIN = 2e-3    # certificate margin for softmin tiles
EX_MARGIN = 5e-4    # certificate margin for exact tiles (bf16 matmul noise)

# Center tiles go through ScalarE softmin, tail tiles through exact DVE
# reduce_max (balances ScalarE ~= DVE; 13 + 19 tiles per direction).
SC_TILES = frozenset(range(8, 21))

bf16 = ml_dtypes.bfloat16

_cache = {}


def band_lo(pt):
    return int(np.clip(pt * PT + PT // 2 - W // 2, 0, N - W))


def _split3(v):
    h = v.astype(bf16)
    m = (v - h.astype(np.float64)).astype(bf16)
    l = (v - h.astype(np.float64) - m.astype(np.float64)).astype(bf16)
    return h, m, l


def build_aug(x, y):
    """x (queries), y (candidates): [3, N] float64 -> AX, AY [128, N] bf16
    with AX[:, i] . AY[:, j] = -(||x_i - y_j||^2), replicated into the four
    32-row PE strips."""
    axh, axm, axl = _split3(2.0 * x)
    yh, ym, yl = _split3(y)
    x2h, x2m, x2l = _split3(-(x * x).sum(0)[None, :])
    y2h, y2m, y2l = _split3(-(y * y).sum(0)[None, :])
    ones = np.ones((3, x.shape[1]), dtype=bf16)
    AX = np.concatenate([
        axh, axh, axm, axh, axl, axm,
        np.concatenate([x2h, x2m, x2l], 0), ones], 0).astype(bf16)
    AY = np.concatenate([
        yh, ym, yh, yl, yh, ym,
        ones, np.concatenate([y2h, y2m, y2l], 0)], 0).astype(bf16)
    n = x.shape[1]
    AX4 = np.zeros((128, n), dtype=bf16)
    AY4 = np.zeros((128, n), dtype=bf16)
    for q in range(4):
        AX4[32 * q:32 * q + K] = AX
        AY4[32 * q:32 * q + K] = AY
    return AX4, AY4


def _build_program():
    import concourse.bass as bass
    import concourse.tile as tile
    from concourse import bacc, mybir

    f32 = mybir.dt.float32
    bf = mybir.dt.bfloat16
    MULT = mybir.AluOpType.mult

    nc = bacc.Bacc(trn_type="TRN2", debug=False, target_bir_lowering=False)

    ax = nc.dram_tensor("ax", [128, N], bf, kind="ExternalInput")
    ay = nc.dram_tensor("ay", [128, N], bf, kind="ExternalInput")
    mu = nc.dram_tensor("mu", [LATENT], f32, kind="ExternalInput")
    lv = nc.dram_tensor("lv", [LATENT], f32, kind="ExternalInput")

    o_row = nc.dram_tensor("o_row", [128, NT], f32, kind="ExternalOutput")
    o_col = nc.dram_tensor("o_col", [128, NT], f32, kind="ExternalOutput")
    o_kl = nc.dram_tensor("o_kl", [128, 1], f32, kind="ExternalOutput")

    with tile.TileContext(nc) as tc, ExitStack() as ctx:
        const = ctx.enter_context(tc.tile_pool(name="const", bufs=1))
        work = ctx.enter_context(tc.tile_pool(name="work", bufs=1))
        stg = ctx.enter_context(tc.tile_pool(name="stg", bufs=4))
        psum_s = ctx.enter_context(
            tc.tile_pool(name="psum_s", bufs=4, space="PSUM"))
        psum_d = ctx.enter_context(
            tc.tile_pool(name="psum_d", bufs=4, space="PSUM"))

        axs = const.tile([128, N], bf, tag="axs")
        ays = const.tile([128, N], bf, tag="ays")
        # Only PE strips 0 (ScalarE tiles) and 2 (DVE tiles) are used, so
        # only partition rows 0-31 / 64-95 are transferred, in column
        # halves so early tiles only wait on the chunk they read.
        CH = 2048
        for k in (0, 1):
            sl = slice(k * CH, (k + 1) * CH)
            nc.sync.dma_start(axs[0:32, sl], ax.ap()[0:32, sl])
            nc.sync.dma_start(axs[64:96, sl], ax.ap()[64:96, sl])
        for k in (0, 1):
            sl = slice(k * CH, (k + 1) * CH)
            nc.gpsimd.dma_start(ays[0:32, sl], ay.ap()[0:32, sl])
            nc.gpsimd.dma_start(ays[64:96, sl], ay.ap()[64:96, sl])

        row_t = const.tile([128, NT], f32, tag="row_t")
        col_t = const.tile([128, NT], f32, tag="col_t")

        # ---- KL term: inside the input-DMA wait window ----
        mu2d = work.tile([128, LATENT // 128], f32, tag="mu2d")
        lv2d = work.tile([128, LATENT // 128], f32, tag="lv2d")
        nc.scalar.dma_start(mu2d[:], mu.ap().rearrange("(p f) -> p f", p=128))
        nc.scalar.dma_start(lv2d[:], lv.ap().rearrange("(p f) -> p f", p=128))
        klsq = work.tile([128, LATENT // 128], f32, tag="klsq")
        klex = work.tile([128, LATENT // 128], f32, tag="klex")
        klt = work.tile([128, LATENT // 128], f32, tag="klt")
        klp = work.tile([128, 1], f32, tag="klp")
        nc.vector.tensor_tensor(klsq[:], mu2d[:], mu2d[:], op=MULT)
        nc.scalar.activation(klex[:], lv2d[:], mybir.ActivationFunctionType.Exp)
        nc.vector.tensor_tensor(klt[:], lv2d[:], klsq[:],
                                op=mybir.AluOpType.subtract)
        nc.vector.tensor_tensor(klt[:], klt[:], klex[:],
                                op=mybir.AluOpType.subtract)
        nc.vector.reduce_sum(klp[:], klt[:], axis=mybir.AxisListType.X)
        nc.sync.dma_start(o_kl.ap(), klp[:])

        # ---- main loop: 2 directions x 32 single-bank band tiles.  Each
        # tile is one row-packed matmul + one evacuation.  ScalarE tiles
        # (PE strip 0) and DVE tiles (PE strip 2) rotate through separate
        # 4-deep single-bank psum pools, so the two evacuation streams
        # are fully decoupled and each engine runs at its own rate; the
        # two streams interleave by estimated finish time.  The y
        # direction reuses the same operands with the roles swapped:
        # AY stationary / AX moving gives -(d(y_i, x_j)). ----
        sc_list = sorted(SC_TILES)
        dv_list = [pt for pt in range(NT) if pt not in SC_TILES]
        order = []
        ts = td = 0.0
        si = vi = 0
        while si < len(sc_list) or vi < len(dv_list):
            if vi >= len(dv_list) or (si < len(sc_list)
                                      and ts + 1.05 <= td + 0.70):
                order.append(("S", sc_list[si])); si += 1; ts += 1.05
            else:
                order.append(("D", dv_list[vi])); vi += 1; td += 0.70

        for di, (stat, mov, ost) in enumerate(
                ((axs, ays, row_t), (ays, axs, col_t))):
            for kind, pt in order:
                lo = band_lo(pt)
                q = 0 if kind == "S" else 2
                pool = psum_s if kind == "S" else psum_d
                ptile = pool.tile([128, W], f32, tag=f"pb{kind}",
                                  name=f"pt{di}_{pt}")
                nc.tensor.matmul(
                    ptile[:],
                    stat[32 * q:32 * q + K, pt * PT:(pt + 1) * PT],
                    mov[32 * q:32 * q + K, lo:lo + W],
                    start=True, stop=True,
                    tile_position=(32 * q, 0),
                )
                if kind == "S":
                    ex = stg.tile([128, W], bf, tag="exh",
                                  name=f"ex{di}_{pt}")
                    nc.scalar.activation(
                        ex[:], ptile[:],
                        mybir.ActivationFunctionType.Exp, scale=S,
                        accum_out=ost[:, pt:pt + 1])
                else:
                    nc.vector.reduce_max(
                        ost[:, pt:pt + 1], ptile[:],
                        axis=mybir.AxisListType.X)
            if di == 0:
                # x-direction results final: ship while y-direction computes
                nc.sync.dma_start(o_row.ap(), row_t[:])
        nc.gpsimd.dma_start(o_col.ap(), col_t[:])

    nc.compile()
    return nc


def _get_nc():
    if "nc" not in _cache:
        _cache["nc"] = _build_program()
    return _cache["nc"]


def _register_ntff_hook():
    import sys, types
    if "antenv.axon_hooks" in sys.modules:
        return
    try:
        from trn_agent_boot.trn_boot import _ntff_profile_via_ctypes
        hook = _ntff_profile_via_ctypes("/opt/axon/libaxon_pjrt.so")
        mod = types.ModuleType("antenv.axon_hooks")
        mod.get_axon_ntff_profile_hook = lambda: hook
        mod.set_axon_ntff_profile_hook = lambda h: None
        sys.modules["antenv.axon_hooks"] = mod
        from concourse import bass_utils
        bass_utils.upload_artifacts = lambda tmpdir: tmpdir
    except Exception:
        pass


def _run(in_maps, trace=False):
    from concourse.bass_utils import run_bass_kernel_spmd
    if trace:
        _register_ntff_hook()
    nc = _get_nc()
    return run_bass_kernel_spmd(nc, in_maps, list(range(NCORES)), trace=trace)


def _side_vals(dev, xs_raw, ys_raw):
    """Decode one direction for one core.

    dev: [128, NT] device output (softmin rowsum for SC_TILES columns,
    -min for the rest).  xs_raw/ys_raw: [3, N] fp32 query/candidate points
    (unsorted).  Returns the mean of per-query-row min squared distances.
    """
    zx = np.argsort(xs_raw[2], kind="stable")
    zy = np.argsort(ys_raw[2], kind="stable")
    xs = xs_raw[:, zx].astype(np.float64)
    ys = ys_raw[:, zy].astype(np.float64)
    thresh = np.exp(LN_THRESH)
    vals = np.zeros(N)
    need = np.zeros(N, dtype=bool)
    dev = dev.astype(np.float64)
    for pt in range(NT):
        rows = slice(pt * PT, pt * PT + PT)
        lo = band_lo(pt)
        hi = lo + W
        zlo = ys[2, lo - 1] if lo > 0 else -np.inf
        zhi = ys[2, hi] if hi < N else np.inf
        zi = xs[2, rows]
        gap = np.minimum(zi - zlo, zhi - zi)
        gap2 = np.where(gap > 0, gap * gap, 0.0)
        v = dev[:, pt]
        if pt in SC_TILES:
            with np.errstate(divide="ignore"):
                est = np.where(v > 0, -np.log(np.maximum(v, 1e-300)) / S,
                               np.inf)
            bad = (v < thresh) | (est > gap2 - SC_MARGIN)
        else:
            est = -v
            bad = est > gap2 - EX_MARGIN
        vals[rows] = est
        need[rows] = bad
    if need.any():
        idx = np.nonzero(need)[0]
        xf = xs.astype(np.float32)
        yf = ys.astype(np.float32)
        xr = xf[:, idx]
        d = ((xr * xr).sum(0)[:, None] + (yf * yf).sum(0)[None, :]
             - 2.0 * xr.T @ yf)
        vals[idx] = d.min(1).astype(np.float64)
    return vals.mean()


def _combine(results, recon_x, x):
    """Host-side finish: decode per-tile reductions, certify bands, rescue."""
    row_total = 0.0
    col_total = 0.0
    kl_sum = 0.0
    for c in range(NCORES):
        r = results[c]
        row_total += _side_vals(r["o_row"], recon_x[c], x[c])
        col_total += _side_vals(r["o_col"], x[c], recon_x[c])
        kl_sum += r["o_kl"].astype(np.float64).sum()

    recon = (row_total + col_total) / NCORES
    kld = -0.5 * (B * LATENT * 1.0 + kl_sum) / B
    total = recon + BETA * kld
    return (np.float32(total), np.float32(recon), np.float32(kld))


def _prep_in_maps(recon_x, x, mu, logvar):
    in_maps = []
    for c in range(NCORES):
        xs = recon_x[c][:, np.argsort(recon_x[c, 2], kind="stable")]
        ys = x[c][:, np.argsort(x[c, 2], kind="stable")]
        xs = xs.astype(np.float64)
        ys = ys.astype(np.float64)
        AX, AY = build_aug(xs, ys)
        in_maps.append({"ax": AX, "ay": AY, "mu": mu[c], "lv": logvar[c]})
    return in_maps


def kernel(recon_x, x, mu, logvar, _trace=False):
    recon_x = np.ascontiguousarray(recon_x, dtype=np.float32)
    x = np.ascontiguousarray(x, dtype=np.float32)
    mu = np.ascontiguousarray(mu, dtype=np.float32)
    logvar = np.ascontiguousarray(logvar, dtype=np.float32)
    in_maps = _prep_in_maps(recon_x, x, mu, logvar)
    res = _run(in_maps, trace=_trace)
    out = _combine(res.results, recon_x, x)
    if _trace:
        return out, res
    return out


# revision 18
# speedup vs baseline: 1.3007x; 1.0039x over previous
"""Trainium2 Bass kernel for AetherLoss: chamfer(recon_x, x) + beta*KL(mu, logvar).

Strategy ("banded KNN", data-parallel over batch B=8 across 8 NeuronCores):

Host prep: both point clouds are sorted by their z coordinate; the fp32->
3x-bf16 augmented operands are built for BOTH directions (AX/AY for
x-query-vs-y and BX/BY for y-query-vs-x), replicated into the four 32-row
PE strips, [128, 4096] bf16 each.

Instead of the full 4096x4096 distance matrix, each 128-query tile only
computes distances to a W=512 window of candidates centered at its sorted
position (one PSUM bank per tile; groups of 4 tiles run as concurrent
row-packed K=24 matmuls in the 4 PE strips).  That is 8x less PSUM traffic
than the all-pairs baseline; per-row nearest-neighbor correctness outside
the band is certified on the host with an exact geometric bound (in-band
min <= squared z-gap to the band edge implies no outside point can win),
and rows failing the certificate (~25%) are recomputed exactly on the host
- the same rescue machinery (and a similar rescue share) as the all-pairs
softmin baseline this replaces.

Per tile the row reduction is either an exact DVE reduce_max over the
negated distances (tail tiles, fp32, no softmin bias) or a ScalarE
exp(S*-d) ACTIVATE whose fused accumulator emits the softmin row sum in
the same pass (center tiles) - the mix balances ScalarE ~= DVE.  The
column direction is handled by the transposed (BX/BY) tiles the same way,
so the all-pairs design's column accumulators, their memsets, and the
per-half DVE max folds disappear entirely; outputs shrink from ~3MB to
33KB per core.  KL runs inside the input-DMA wait window at boot; the
x-direction results are DMA-ed out at the loop midpoint.

Host combine (numpy, float64): exact tiles give -min directly; softmin
tiles give -ln(rowsum)/S with S=1600; rows with rowsum underflow or a
failed band certificate are recomputed exactly from the raw inputs (one
small fp32 gemm per core per direction).  Validated end-to-end in numpy
against the fp32 reference at rel err ~1.7e-4 (matching the all-pairs
baseline's measured error).
"""

import numpy as np
import ml_dtypes
from contextlib import ExitStack

B, D, N = 8, 3, 4096
LATENT = 256
NCORES = 8
BETA = 1.0

K = 24              # augmented contraction size
PT = 128            # query tile size
NT = N // PT        # 32 query tiles per direction
W = 512             # candidate band width (1 PSUM bank)

S = 1600.0          # softmin sharpness
LN_THRESH = -60.0   # underflow threshold on ln(rowsum)
SC_MARGIN = 2e-3    # certificate margin for softmin tiles
EX_MARGIN = 5e-4    # certificate margin for exact tiles (bf16 matmul noise)

# Center tiles go through ScalarE softmin, tail tiles through exact DVE
# reduce_max (balances ScalarE ~= DVE; 12 + 20 tiles per direction).
SC_TILES = frozenset(range(8, 20))

bf16 = ml_dtypes.bfloat16

_cache = {}


def band_lo(pt):
    return int(np.clip(pt * PT + PT // 2 - W // 2, 0, N - W))


def _split3(v):
    h = v.astype(bf16)
    m = (v - h.astype(np.float64)).astype(bf16)
    l = (v - h.astype(np.float64) - m.astype(np.float64)).astype(bf16)
    return h, m, l


def build_aug(x, y):
    """x (queries), y (candidates): [3, N] float64 -> AX, AY [128, N] bf16
    with AX[:, i] . AY[:, j] = -(||x_i - y_j||^2), replicated into the four
    32-row PE strips."""
    axh, axm, axl = _split3(2.0 * x)
    yh, ym, yl = _split3(y)
    x2h, x2m, x2l = _split3(-(x * x).sum(0)[None, :])
    y2h, y2m, y2l = _split3(-(y * y).sum(0)[None, :])
    ones = np.ones((3, x.shape[1]), dtype=bf16)
    AX = np.concatenate([
        axh, axh, axm, axh, axl, axm,
        np.concatenate([x2h, x2m, x2l], 0), ones], 0).astype(bf16)
    AY = np.concatenate([
        yh, ym, yh, yl, yh, ym,
        ones, np.concatenate([y2h, y2m, y2l], 0)], 0).astype(bf16)
    n = x.shape[1]
    AX4 = np.zeros((128, n), dtype=bf16)
    AY4 = np.zeros((128, n), dtype=bf16)
    for q in range(4):
        AX4[32 * q:32 * q + K] = AX
        AY4[32 * q:32 * q + K] = AY
    return AX4, AY4


def _build_program():
    import concourse.bass as bass
    import concourse.tile as tile
    from concourse import bacc, mybir

    f32 = mybir.dt.float32
    bf = mybir.dt.bfloat16
    MULT = mybir.AluOpType.mult

    nc = bacc.Bacc(trn_type="TRN2", debug=False, target_bir_lowering=False)

    ax = nc.dram_tensor("ax", [128, N], bf, kind="ExternalInput")
    ay = nc.dram_tensor("ay", [128, N], bf, kind="ExternalInput")
    mu = nc.dram_tensor("mu", [LATENT], f32, kind="ExternalInput")
    lv = nc.dram_tensor("lv", [LATENT], f32, kind="ExternalInput")

    o_row = nc.dram_tensor("o_row", [128, NT], f32, kind="ExternalOutput")
    o_col = nc.dram_tensor("o_col", [128, NT], f32, kind="ExternalOutput")
    o_kl = nc.dram_tensor("o_kl", [128, 1], f32, kind="ExternalOutput")

    with tile.TileContext(nc) as tc, ExitStack() as ctx:
        const = ctx.enter_context(tc.tile_pool(name="const", bufs=1))
        work = ctx.enter_context(tc.tile_pool(name="work", bufs=1))
        stg = ctx.enter_context(tc.tile_pool(name="stg", bufs=4))
        psum_s = ctx.enter_context(
            tc.tile_pool(name="psum_s", bufs=4, space="PSUM"))
        psum_d = ctx.enter_context(
            tc.tile_pool(name="psum_d", bufs=2, space="PSUM"))

        axs = const.tile([128, N], bf, tag="axs")
        ays = const.tile([128, N], bf, tag="ays")
        # Only PE strips 0 (ScalarE tiles) and 2,3 (DVE pairs) are used,
        # so only partition rows 0-31 / 64-127 are transferred, in column
        # halves so early tiles only wait on the chunk they read.  The
        # DVE-pair slices go first: the first issued tiles need them.
        CH = 2048
        for k in (0, 1):
            sl = slice(k * CH, (k + 1) * CH)
            nc.sync.dma_start(axs[64:128, sl], ax.ap()[64:128, sl])
            nc.sync.dma_start(axs[0:32, sl], ax.ap()[0:32, sl])
        for k in (0, 1):
            sl = slice(k * CH, (k + 1) * CH)
            nc.gpsimd.dma_start(ays[64:128, sl], ay.ap()[64:128, sl])
            nc.gpsimd.dma_start(ays[0:32, sl], ay.ap()[0:32, sl])

        row_t = const.tile([128, NT], f32, tag="row_t")
        col_t = const.tile([128, NT], f32, tag="col_t")

        # ---- KL term: inside the input-DMA wait window ----
        mu2d = work.tile([128, LATENT // 128], f32, tag="mu2d")
        lv2d = work.tile([128, LATENT // 128], f32, tag="lv2d")
        nc.scalar.dma_start(mu2d[:], mu.ap().rearrange("(p f) -> p f", p=128))
        nc.scalar.dma_start(lv2d[:], lv.ap().rearrange("(p f) -> p f", p=128))
        klsq = work.tile([128, LATENT // 128], f32, tag="klsq")
        klex = work.tile([128, LATENT // 128], f32, tag="klex")
        klt = work.tile([128, LATENT // 128], f32, tag="klt")
        klp = work.tile([128, 1], f32, tag="klp")
        nc.vector.tensor_tensor(klsq[:], mu2d[:], mu2d[:], op=MULT)
        nc.scalar.activation(klex[:], lv2d[:], mybir.ActivationFunctionType.Exp)
        nc.vector.tensor_tensor(klt[:], lv2d[:], klsq[:],
                                op=mybir.AluOpType.subtract)
        nc.vector.tensor_tensor(klt[:], klt[:], klex[:],
                                op=mybir.AluOpType.subtract)
        nc.vector.reduce_sum(klp[:], klt[:], axis=mybir.AxisListType.X)
        nc.sync.dma_start(o_kl.ap(), klp[:])

        # ---- main loop: 2 directions x 32 band tiles.  ScalarE tiles
        # (PE strip 0, single-bank psum, exp+accum softmin) and DVE tile
        # pairs (PE strips 2+3, two-bank psum, one [128,2,W] reduce_max)
        # rotate through separate psum pools, so the two evacuation
        # streams are fully decoupled and each engine runs at its own
        # rate; the streams interleave by estimated finish time.  The y
        # direction reuses the same operands with the roles swapped:
        # AY stationary / AX moving gives -(d(y_i, x_j)). ----
        sc_list = sorted(SC_TILES)
        dv_list = [pt for pt in range(NT) if pt not in SC_TILES]
        dv_pairs = [(dv_list[2 * i], dv_list[2 * i + 1])
                    for i in range(len(dv_list) // 2)]
        assert all(b == a + 1 for a, b in dv_pairs)
        order = []
        ts = td = 0.0
        si = vi = 0
        while si < len(sc_list) or vi < len(dv_pairs):
            if vi >= len(dv_pairs) or (si < len(sc_list)
                                       and ts + 0.95 <= td + 1.25):
                order.append(("S", sc_list[si])); si += 1; ts += 0.95
            else:
                order.append(("D", dv_pairs[vi])); vi += 1; td += 1.25

        for di, (stat, mov, ost) in enumerate(
                ((axs, ays, row_t), (ays, axs, col_t))):
            for kind, item in order:
                if kind == "S":
                    pt = item
                    ptile = psum_s.tile([128, W], f32, tag="pbS",
                                        name=f"pt{di}_{pt}")
                    lo = band_lo(pt)
                    nc.tensor.matmul(
                        ptile[:],
                        stat[0:K, pt * PT:(pt + 1) * PT],
                        mov[0:K, lo:lo + W],
                        start=True, stop=True,
                        tile_position=(0, 0),
                    )
                    ex = stg.tile([128, W], bf, tag="exh",
                                  name=f"ex{di}_{pt}")
                    nc.scalar.activation(
                        ex[:], ptile[:],
                        mybir.ActivationFunctionType.Exp, scale=S,
                        accum_out=ost[:, pt:pt + 1])
                else:
                    pa, pb = item
                    ptile = psum_d.tile([128, 2 * W], f32, tag="pbD",
                                        name=f"pt{di}_{pa}")
                    for j, pt in enumerate(item):
                        q = 2 + j
                        lo = band_lo(pt)
                        nc.tensor.matmul(
                            ptile[:, j * W:(j + 1) * W],
                            stat[32 * q:32 * q + K, pt * PT:(pt + 1) * PT],
                            mov[32 * q:32 * q + K, lo:lo + W],
                            start=True, stop=True,
                            tile_position=(32 * q, 0),
                        )
                    nc.vector.reduce_max(
                        ost[:, pa:pa + 2],
                        ptile[:].rearrange("p (m w) -> p m w", m=2),
                        axis=mybir.AxisListType.X)
            if di == 0:
                # x-direction results final: ship while y-direction computes
                nc.sync.dma_start(o_row.ap(), row_t[:])
        nc.sync.dma_start(o_col.ap(), col_t[:])

    nc.compile()
    return nc


def _get_nc():
    if "nc" not in _cache:
        _cache["nc"] = _build_program()
    return _cache["nc"]


def _register_ntff_hook():
    import sys, types
    if "antenv.axon_hooks" in sys.modules:
        return
    try:
        from trn_agent_boot.trn_boot import _ntff_profile_via_ctypes
        hook = _ntff_profile_via_ctypes("/opt/axon/libaxon_pjrt.so")
        mod = types.ModuleType("antenv.axon_hooks")
        mod.get_axon_ntff_profile_hook = lambda: hook
        mod.set_axon_ntff_profile_hook = lambda h: None
        sys.modules["antenv.axon_hooks"] = mod
        from concourse import bass_utils
        bass_utils.upload_artifacts = lambda tmpdir: tmpdir
    except Exception:
        pass


def _run(in_maps, trace=False):
    from concourse.bass_utils import run_bass_kernel_spmd
    if trace:
        _register_ntff_hook()
    nc = _get_nc()
    return run_bass_kernel_spmd(nc, in_maps, list(range(NCORES)), trace=trace)


def _side_vals(dev, xs_raw, ys_raw):
    """Decode one direction for one core.

    dev: [128, NT] device output (softmin rowsum for SC_TILES columns,
    -min for the rest).  xs_raw/ys_raw: [3, N] fp32 query/candidate points
    (unsorted).  Returns the mean of per-query-row min squared distances.
    """
    zx = np.argsort(xs_raw[2], kind="stable")
    zy = np.argsort(ys_raw[2], kind="stable")
    xs = xs_raw[:, zx].astype(np.float64)
    ys = ys_raw[:, zy].astype(np.float64)
    thresh = np.exp(LN_THRESH)
    vals = np.zeros(N)
    need = np.zeros(N, dtype=bool)
    dev = dev.astype(np.float64)
    for pt in range(NT):
        rows = slice(pt * PT, pt * PT + PT)
        lo = band_lo(pt)
        hi = lo + W
        zlo = ys[2, lo - 1] if lo > 0 else -np.inf
        zhi = ys[2, hi] if hi < N else np.inf
        zi = xs[2, rows]
        gap = np.minimum(zi - zlo, zhi - zi)
        gap2 = np.where(gap > 0, gap * gap, 0.0)
        v = dev[:, pt]
        if pt in SC_TILES:
            with np.errstate(divide="ignore"):
                est = np.where(v > 0, -np.log(np.maximum(v, 1e-300)) / S,
                               np.inf)
            bad = (v < thresh) | (est > gap2 - SC_MARGIN)
        else:
            est = -v
            bad = est > gap2 - EX_MARGIN
        vals[rows] = est
        need[rows] = bad
    if need.any():
        idx = np.nonzero(need)[0]
        xf = xs.astype(np.float32)
        yf = ys.astype(np.float32)
        xr = xf[:, idx]
        d = ((xr * xr).sum(0)[:, None] + (yf * yf).sum(0)[None, :]
             - 2.0 * xr.T @ yf)
        vals[idx] = d.min(1).astype(np.float64)
    return vals.mean()


def _combine(results, recon_x, x):
    """Host-side finish: decode per-tile reductions, certify bands, rescue."""
    row_total = 0.0
    col_total = 0.0
    kl_sum = 0.0
    for c in range(NCORES):
        r = results[c]
        row_total += _side_vals(r["o_row"], recon_x[c], x[c])
        col_total += _side_vals(r["o_col"], x[c], recon_x[c])
        kl_sum += r["o_kl"].astype(np.float64).sum()

    recon = (row_total + col_total) / NCORES
    kld = -0.5 * (B * LATENT * 1.0 + kl_sum) / B
    total = recon + BETA * kld
    return (np.float32(total), np.float32(recon), np.float32(kld))


def _prep_in_maps(recon_x, x, mu, logvar):
    in_maps = []
    for c in range(NCORES):
        xs = recon_x[c][:, np.argsort(recon_x[c, 2], kind="stable")]
        ys = x[c][:, np.argsort(x[c, 2], kind="stable")]
        xs = xs.astype(np.float64)
        ys = ys.astype(np.float64)
        AX, AY = build_aug(xs, ys)
        in_maps.append({"ax": AX, "ay": AY, "mu": mu[c], "lv": logvar[c]})
    return in_maps


def kernel(recon_x, x, mu, logvar, _trace=False):
    recon_x = np.ascontiguousarray(recon_x, dtype=np.float32)
    x = np.ascontiguousarray(x, dtype=np.float32)
    mu = np.ascontiguousarray(mu, dtype=np.float32)
    logvar = np.ascontiguousarray(logvar, dtype=np.float32)
    in_maps = _prep_in_maps(recon_x, x, mu, logvar)
    res = _run(in_maps, trace=_trace)
    out = _combine(res.results, recon_x, x)
    if _trace:
        return out, res
    return out
